# revision 36
# baseline (speedup 1.0000x reference)
"""Trainium2 Bass kernel for nn_CaevlFT_39367670235990 (retrieval_knn VICReg-style loss).

Strategy (2 SPMD launches over 8 cores, no collectives):
  Launch 1 (batch-sharded, 32 samples/core): the per-sample feature
    distance-dot matrices D[s] = M1[s]^T M2[s] (49x49, contraction over
    C=512) as bf16 matmuls. Output: all D matrices (307KB/core).
  Host: argmins (feature + location branches), rank selection, gathers,
    per-(pair,side) centering, bf16 packing; invariance terms extracted
    algebraically from D (|a|^2 + |b|^2 - 2 D[i, j*]).
  Launch 2 (m-sharded, 16 pair-slots/core): per-channel sumsq (variance
    + cov diag) and the 256x256 batch Gram G = Xc^T Xc (contraction over
    C) per pair-side via the identity ||X^T X||_F = ||X X^T||_F.
    Global embedding: per-core partial Grams over 1024 channels, output
    raw (host sums across cores before squaring).
  Host: scalar epilogue.

All shapes hardcoded for B=256, C=512, HW=49, D=8192, 8 cores.
"""

import os
import sys
import numpy as np

for p in ("/opt/trn_rl_repo", "/opt/pypackages"):
    if p not in sys.path:
        sys.path.insert(0, p)

import ml_dtypes

BF16 = ml_dtypes.bfloat16
FP8 = bool(os.environ.get("KERNEL_FP8"))
DR = bool(os.environ.get("KERNEL_DR"))
FP8P1 = bool(os.environ.get("KERNEL_FP8P1"))
P2DT = ml_dtypes.float8_e4m3 if FP8 else BF16  # phase-2 feature dtype
P1DT = ml_dtypes.float8_e4m3 if FP8P1 else BF16  # phase-1 map dtype

NCORES = 8
B = 256
BL = B // NCORES          # 32 samples per core in launch 1
C = 512
HW = 49
D = 8192
EPS = 1e-5
NPAIR = 16                # pair slots per core in launch 2 (122 real + 6 pad)
GCH = D // NCORES // 128  # 8 chunks of 128 channels per core (global branch)

_SIM = bool(os.environ.get("KERNEL_SIM"))


# ----------------------------------------------------------------------------
# Launch 1: per-sample distance dot matrices (batch-sharded)
# ----------------------------------------------------------------------------
def build_phase1():
    import concourse.bass as bass
    import concourse.bacc as bacc
    import concourse.tile as tile
    from concourse import mybir

    F32 = mybir.dt.float32
    BF = mybir.dt.bfloat16

    nc = bacc.Bacc("TRN2", target_bir_lowering=False, debug=False,
                   enable_asserts=False, num_devices=NCORES)
    PD = mybir.dt.float8e4 if FP8P1 else BF
    # mb[p, s, which, f]: m1 (which=0) and m2 (which=1) interleaved so each
    # chunk needs a single DMA
    mb = nc.dram_tensor("mb", [128, BL, 2, 196], PD, kind="ExternalInput").ap()
    d_o = nc.dram_tensor("d_o", [49, BL, 49], F32, kind="ExternalOutput").ap()

    # ramp-up chunk sizes: small first chunks for early PE start, big later
    # chunks for DMA descriptor efficiency
    CHUNKS = [2, 4, 26]
    with tile.TileContext(nc) as tc:
        with (
            tc.tile_pool(name="big", bufs=1) as big,
            tc.tile_pool(name="xin", bufs=3) as xin,
            tc.tile_pool(name="pd", bufs=8, space=bass.MemorySpace.PSUM) as pd,
        ):
            Dall = big.tile([49, BL, 49], F32, tag="Dall")
            s0 = 0
            for ci, ch in enumerate(CHUNKS):
                T = xin.tile([128, ch, 2, 196], PD, tag=f"T{ch}",
                             name=f"T_{s0}")
                eng = nc.sync if ci % 2 == 0 else nc.scalar
                eng.dma_start(T[:], mb[:, s0:s0 + ch])
                for sl in range(0, ch, 2):
                    s = s0 + sl
                    Dp = pd.tile([49, 2, 49], F32, tag="dmat", name=f"Dp_{s}")
                    for j in range(2):
                        for q in range(4):
                            nc.tensor.matmul(
                                Dp[:, j, :],
                                T[:, sl + j, 0, q * 49:(q + 1) * 49],
                                T[:, sl + j, 1, q * 49:(q + 1) * 49],
                                start=(q == 0), stop=(q == 3))
                    nc.vector.tensor_copy(Dall[:, s:s + 2, :], Dp[:])
                s0 += ch
            assert s0 == BL
            nc.sync.dma_start(d_o, Dall[:])

    nc.compile()
    return nc


# ----------------------------------------------------------------------------
# Launch 2: cross-batch statistics (pair-sharded)
# ----------------------------------------------------------------------------
def build_phase2():
    import concourse.bass as bass
    import concourse.bacc as bacc
    import concourse.tile as tile
    from concourse import mybir

    F32 = mybir.dt.float32
    BF = mybir.dt.bfloat16
    AX = mybir.AxisListType
    OP = mybir.AluOpType
    AF = mybir.ActivationFunctionType

    nc = bacc.Bacc("TRN2", target_bir_lowering=False, debug=False,
                   enable_asserts=False, num_devices=NCORES)
    PD = mybir.dt.float8e4 if FP8 else BF
    # XP[p, t, side, k, b] = centered feature value of pair t, side,
    # channel k*128+p, batch b
    xp = nc.dram_tensor("xp", [128, NPAIR, 2, 4, 256], PD,
                        kind="ExternalInput").ap()
    # GP[p, side, kc, b]: global embedding chunk (channels kc*128+p of this
    # core's 1024-channel shard), centered
    gp = nc.dram_tensor("gp", [128, 2, GCH, 256], PD, kind="ExternalInput").ap()

    def gram_mms(nc, G, xv, m, nk):
        # accumulate G[:, m*256:(m+1)*256] += xv_chunk^T @ xv over nk chunks
        if DR:
            for kk in range(nk // 2):
                nc.tensor.matmul(
                    G[:, m * 256:(m + 1) * 256],
                    xv[:, 2 * kk:2 * kk + 2, m * 128:(m + 1) * 128],
                    xv[:, 2 * kk:2 * kk + 2, :],
                    start=(kk == 0), stop=(kk == nk // 2 - 1),
                    perf_mode=mybir.MatmulPerfMode.DoubleRow)
        else:
            for k in range(nk):
                nc.tensor.matmul(
                    G[:, m * 256:(m + 1) * 256],
                    xv[:, k, m * 128:(m + 1) * 128],
                    xv[:, k, :], start=(k == 0), stop=(k == nk - 1))

    go = nc.dram_tensor("g_o", [128, NPAIR * 2], F32, kind="ExternalOutput").ap()
    gm_o = nc.dram_tensor("gm_o", [2, 128, 512], F32, kind="ExternalOutput").ap()

    with tile.TileContext(nc) as tc:
        with (
            tc.tile_pool(name="stage", bufs=1) as stage,
            tc.tile_pool(name="xin", bufs=3) as xin,
            tc.tile_pool(name="work", bufs=4) as work,
            tc.tile_pool(name="pg", bufs=3, space=bass.MemorySpace.PSUM) as pg,
        ):
            GO = stage.tile([128, NPAIR * 2], F32, tag="GO")

            # stream pairs in ramped groups, alternating HWDGE queues
            GROUPS = [1, 1, 2, 4, 8]
            g0 = 0
            for gi, grp in enumerate(GROUPS):
                X = xin.tile([128, grp, 2, 4, 256], PD, tag=f"X{grp}",
                             name=f"X_{g0}")
                eng = nc.sync if gi % 2 == 0 else nc.scalar
                eng.dma_start(X[:], xp[:, g0:g0 + grp])
                for tl in range(grp):
                    t = g0 + tl
                    for side in range(2):
                        xv = X[:, tl, side]              # [128, 4, 256]
                        # Gram G = Xc^T Xc over C: one [128,512] psum tile,
                        # free = (m, b'): G[m*128+p, b'] at [p, m*256+b']
                        G = pg.tile([128, 512], F32, tag=f"G{side}",
                                    name=f"G_{t}_{side}")
                        for m in range(2):
                            gram_mms(nc, G, xv, m, 4)
                        # sum G^2 per partition: 2/3 scalar, 1/3 vector
                        gc = t * 2 + side
                        if t % 3 != 2:
                            scr = work.tile([128, 512], F32, tag="scr",
                                            name=f"scr_{t}_{side}")
                            nc.scalar.activation(scr[:], G[:], AF.Square,
                                                 accum_out=GO[:, gc:gc + 1])
                        else:
                            gb = work.tile([128, 512], BF, tag="gb",
                                           name=f"gb_{t}_{side}")
                            nc.vector.tensor_copy(gb[:], G[:])
                            sq = work.tile([128, 512], BF, tag="vsq",
                                           name=f"vsq_{t}_{side}")
                            nc.vector.tensor_tensor(sq[:], gb[:], gb[:],
                                                    OP.mult)
                            nc.vector.tensor_reduce(GO[:, gc:gc + 1], sq[:],
                                                    AX.X, OP.add)
                g0 += grp
            assert g0 == NPAIR

            # global embedding: partial Grams output raw (host sums cores)
            XG = xin.tile([128, 2, GCH, 256], PD, tag="XG")
            nc.sync.dma_start(XG[:], gp)
            for side in range(2):
                xv = XG[:, side]                          # [128, GCH, 256]
                G = pg.tile([128, 512], F32, tag=f"G{side}",
                            name=f"GG_{side}")
                for m in range(2):
                    gram_mms(nc, G, xv, m, GCH)
                gm = work.tile([128, 512], F32, tag="gm",
                               name=f"gmc_{side}")
                nc.vector.tensor_copy(gm[:], G[:])
                nc.sync.dma_start(gm_o[side], gm[:])

            nc.sync.dma_start(go, GO[:])

    nc.compile()
    return nc


_NC1 = None
_NC2 = None


def _get_ncs():
    global _NC1, _NC2
    if _NC1 is None:
        _NC1 = build_phase1()
    if _NC2 is None:
        _NC2 = build_phase2()
    return _NC1, _NC2


# ----------------------------------------------------------------------------
# numpy simulation of the two launches (for host-logic validation)
# ----------------------------------------------------------------------------
def _sim_phase1(in_maps):
    out = []
    for im in in_maps:
        a = im["mb"][:, :, 0].astype(np.float32)   # [128, 32, 196]
        b = im["mb"][:, :, 1].astype(np.float32)
        Dall = np.zeros((49, BL, 49), np.float32)
        for q in range(4):
            Dall += np.einsum("psi,psj->isj", a[:, :, q * 49:(q + 1) * 49],
                              b[:, :, q * 49:(q + 1) * 49])
        out.append({"d_o": Dall})
    return out


def _sim_phase2(in_maps):
    out = []
    for im in in_maps:
        xp = im["xp"].astype(np.float32)   # [128, 16, 2, 4, 256]
        gp = im["gp"].astype(np.float32)   # [128, 2, 8, 256]
        GO = np.zeros((128, NPAIR * 2), np.float32)
        GM = np.zeros((2, 128, 512), np.float32)
        for t in range(NPAIR):
            for side in range(2):
                xv = xp[:, t, side]  # [128, 4, 256]
                for m in range(2):
                    G = np.einsum("pkb,pkc->bc",
                                  xv[:, :, m * 128:(m + 1) * 128], xv)
                    if t % 3 == 2:
                        G = G.astype(BF16).astype(np.float32)
                    GO[:, t * 2 + side] += (G ** 2).sum(-1)
        for side in range(2):
            xv = gp[:, side]      # [128, 8, 256]
            for m in range(2):
                G = np.einsum("pkb,pkc->bc", xv[:, :, m * 128:(m + 1) * 128], xv)
                GM[side, :, m * 256:(m + 1) * 256] = G
        out.append({"g_o": GO, "gm_o": GM})
    return out


# ----------------------------------------------------------------------------
# host orchestration
# ----------------------------------------------------------------------------
def _grid():
    c = (np.arange(7, dtype=np.float32) + 0.5) * (224.0 / 7.0)
    gx = np.repeat(c[:, None], 7, axis=1)
    gy = np.repeat(c[None, :], 7, axis=0)
    return np.stack([gx, gy], axis=-1).reshape(49, 2)  # (49,2)


def kernel(maps_1, maps_2, projected_x, projected_y, locations,
           _return_time=False):
    m1 = np.ascontiguousarray(maps_1.reshape(B, C, HW), np.float32)
    m2 = np.ascontiguousarray(maps_2.reshape(B, C, HW), np.float32)
    loc = np.asarray(locations, np.float32)

    # ---- phase 1: distance dot matrices on device
    m1f = m1.reshape(B, 128, 196)
    m2f = m2.reshape(B, 128, 196)
    in1 = []
    for k in range(NCORES):
        sl = slice(k * BL, (k + 1) * BL)
        # [128, BL, 2, 196]
        comb = np.stack([m1f[sl].transpose(1, 0, 2),
                         m2f[sl].transpose(1, 0, 2)], axis=2)
        in1.append({"mb": np.ascontiguousarray(comb).astype(P1DT)})

    trace = bool(os.environ.get("KBENCH_TRACE"))
    if _SIM:
        r1res, t1 = _sim_phase1(in1), None
    else:
        from concourse.bass_utils import run_bass_kernel_spmd
        nc1, _ = _get_ncs()
        r1 = run_bass_kernel_spmd(nc1, in1, core_ids=list(range(NCORES)),
                                  trace=trace)
        r1res, t1 = r1.results, r1.exec_time_ns

    # D[s, i, j] = <m1[s,:,i], m2[s,:,j]> (bf16 products, f32 accum)
    Dm = np.concatenate([r["d_o"].transpose(1, 0, 2) for r in r1res], 0)

    # ---- host: argmins, selections, invariance
    a2 = np.einsum("bci,bci->bi", m1, m1)          # |a_i|^2  (B, 49)
    b2 = np.einsum("bci,bci->bi", m2, m2)          # |b_j|^2  (B, 49)
    dist1 = a2[:, :, None] + b2[:, None, :] - 2.0 * Dm   # (B, 49, 49)
    nn1 = np.argmin(dist1, axis=2)                 # (B, 49) m1 -> m2
    nn2 = np.argmin(dist1, axis=1)                 # (B, 49) m2 -> m1
    ar = np.arange(B)[:, None]
    inv1 = dist1[ar, np.arange(49)[None, :], nn1].mean(1) / C   # (B,)
    inv2 = dist1[ar, nn2, np.arange(49)[None, :]].mean(1) / C   # (B,)

    g = _grid()
    dl = ((g[None, :, None, :] - loc[:, None, :, :]) ** 2).sum(-1)  # (B,49,49)
    nnL = np.argmin(dl, axis=2)
    nvL = np.min(dl, axis=2)
    nnL2 = np.argmin(dl, axis=1)
    nvL2 = np.min(dl, axis=1)

    def select(nv, k):
        rank = np.argsort(np.argsort(nv, axis=1, kind="stable"),
                          axis=1, kind="stable")
        sel = np.nonzero(rank < k)[1].reshape(B, k)
        return sel

    sel1 = select(nvL, 20)                          # (B, 20) positions in m1
    sel2 = select(nvL2, 4)                          # (B, 4) positions in m2
    nn_s1 = np.take_along_axis(nnL, sel1, axis=1)   # m2 indices
    nn_s2 = np.take_along_axis(nnL2, sel2, axis=1)  # m1 indices

    inv3 = (np.take_along_axis(a2, sel1, 1) + np.take_along_axis(b2, nn_s1, 1)
            - 2.0 * Dm[ar, sel1, nn_s1]).mean(1) / C
    inv4 = (np.take_along_axis(b2, sel2, 1) + np.take_along_axis(a2, nn_s2, 1)
            - 2.0 * Dm[ar, nn_s2, sel2]).mean(1) / C

    # ---- build phase-2 pair list (x, y) as (npairs, B, C)
    m1t = m1.transpose(0, 2, 1)   # (B, 49, C)
    m2t = m2.transpose(0, 2, 1)
    X_parts = [m1t, m2t,
               np.take_along_axis(m1t, sel1[:, :, None], 1),
               np.take_along_axis(m2t, sel2[:, :, None], 1)]
    Y_parts = [np.take_along_axis(m2t, nn1[:, :, None], 1),
               np.take_along_axis(m1t, nn2[:, :, None], 1),
               np.take_along_axis(m2t, nn_s1[:, :, None], 1),
               np.take_along_axis(m1t, nn_s2[:, :, None], 1)]
    Xl = np.concatenate(X_parts, 1).transpose(1, 0, 2)   # (122, B, C)
    Yl = np.concatenate(Y_parts, 1).transpose(1, 0, 2)
    NP = NCORES * NPAIR
    PA = np.zeros((NP, 2, B, C), np.float32)
    PA[:122, 0] = Xl
    PA[:122, 1] = Yl
    PA -= PA.mean(2, keepdims=True)
    PAb = PA.astype(P2DT)
    # device layout [core][128, NPAIR, 2, 4, 256]:
    #   [p, t, side, k, b] = PAb[core*16+t, side, b, k*128+p]
    PAr = PAb.transpose(0, 1, 3, 2).reshape(NCORES, NPAIR, 2, 4, 128, 256)
    PAr = np.ascontiguousarray(PAr.transpose(0, 4, 1, 2, 3, 5))

    px = np.asarray(projected_x, np.float32)
    py = np.asarray(projected_y, np.float32)
    pxc = (px - px.mean(0, keepdims=True)).T.astype(P2DT)   # (D, B)
    pyc = (py - py.mean(0, keepdims=True)).T.astype(P2DT)
    pxr = pxc.reshape(NCORES, GCH, 128, 256).transpose(0, 2, 1, 3)
    pyr = pyc.reshape(NCORES, GCH, 128, 256).transpose(0, 2, 1, 3)

    in2 = []
    for k in range(NCORES):
        in2.append({
            "xp": PAr[k],
            "gp": np.ascontiguousarray(
                np.stack([pxr[k], pyr[k]], 1)),   # [128, 2, GCH, 256]
        })

    if _SIM:
        r2res, t2 = _sim_phase2(in2), None
    else:
        from concourse.bass_utils import run_bass_kernel_spmd
        _, nc2 = _get_ncs()
        r2 = run_bass_kernel_spmd(nc2, in2, core_ids=list(range(NCORES)),
                                  trace=trace)
        r2res, t2 = r2.results, r2.exec_time_ns

    # ---- host epilogue
    # per-(pair,side) channel sumsq from the same bf16 values the device saw
    ssq = (PAb.astype(np.float32) ** 2).sum(2).astype(np.float64)  # (NP,2,C)

    def pair_stats(pidx):
        k, t = divmod(pidx, NPAIR)
        res = r2res[k]
        gsum = res["g_o"][:, t * 2:t * 2 + 2].astype(np.float64).sum(0)
        return ssq[pidx, 0], ssq[pidx, 1], gsum[0], gsum[1]

    def relu_std_sum(s):
        # s = per-channel sumsq of centered bf16 (any shape); returns
        # sum over channels of relu(1 - sqrt(var + eps))
        std = np.sqrt(s / (B - 1) + EPS)
        return np.maximum(1.0 - std, 0.0).sum()

    # pair index ranges: L1a: 0-48, L1b: 49-97, L2a: 98-117, L2b: 118-121
    spans = {"L1a": (0, 49), "L1b": (49, 98), "L2a": (98, 118),
             "L2b": (118, 122)}
    stdsum = {}
    offd = {}
    for tag, (lo, hi) in spans.items():
        ss = 0.0
        od = 0.0
        for pidx in range(lo, hi):
            sx, sy, gx, gy = pair_stats(pidx)
            ss += relu_std_sum(sx) + relu_std_sum(sy)
            od += ((gx - (sx ** 2).sum()) / 2 + (gy - (sy ** 2).sum()) / 2) \
                / ((B - 1.0) ** 2)
        stdsum[tag] = ss
        offd[tag] = od

    def loss_maps(tag, inv, M):
        inv_t = 25.0 * inv
        std_t = 25.0 * stdsum[tag] / (2.0 * M * C)
        cov_t = 1.0 * offd[tag] / C / M
        return inv_t, std_t, cov_t

    i1, s1, c1 = loss_maps("L1a", inv1, 49)
    i2, s2, c2 = loss_maps("L1b", inv2, 49)
    i3, s3, c3 = loss_maps("L2a", inv3, 20)
    i4, s4, c4 = loss_maps("L2b", inv4, 4)
    local = ((i1 + i2) / 2 + (s1 + s2) / 2 + (c1 + c2) / 2
             + (i3 + i4) / 2 + (s3 + s4) / 2 + (c3 + c4) / 2)

    # global embedding loss
    Gx = np.zeros((256, 256), np.float64)
    Gy = np.zeros((256, 256), np.float64)
    for k in range(NCORES):
        res = r2res[k]
        gm = res["gm_o"].astype(np.float64)  # [2, 128, 512]
        Gx += np.concatenate([gm[0, :, 0:256], gm[0, :, 256:512]], 0)
        Gy += np.concatenate([gm[1, :, 0:256], gm[1, :, 256:512]], 0)
    gsx = (pxc.astype(np.float32) ** 2).sum(1).astype(np.float64)  # (D,)
    gsy = (pyc.astype(np.float32) ** 2).sum(1).astype(np.float64)
    sx2 = (gsx ** 2).sum()
    sy2 = (gsy ** 2).sum()
    rgx = relu_std_sum(gsx)
    rgy = relu_std_sum(gsy)
    inv_g = ((px - py) ** 2).mean(1)
    std_g = rgx / D / 2 + rgy / D / 2
    offd_gx = ((Gx ** 2).sum() - sx2) / ((B - 1.0) ** 2)
    offd_gy = ((Gy ** 2).sum() - sy2) / ((B - 1.0) ** 2)
    cov_g = offd_gx / D + offd_gy / D
    glob = 25.0 * inv_g + 25.0 * std_g + 1.0 * cov_g

    out = (0.5 * glob + 0.5 * local).astype(np.float32)
    if _return_time:
        return out, (t1, t2)
    return out
